# revision 18
# baseline (speedup 1.0000x reference)
"""Trainium2 Bass kernel for MoE-LoRA GQA attention (nn_Attention_57389353009692).

Strategy (8 NeuronCores, one SPMD launch):
  - Tensor-parallel over heads: core c owns q-heads 4c..4c+3 and kv-head c.
  - Interleaved pipeline: for each 512-token block i: QKV projections
    (+MoE-LoRA, RoPE) for block i, then flash attention for query block i
    over key tiles 0..4i+3. Keeps the PE dense (projection matmuls fill
    the windows where attention waits on exp) so the HAM clock gate stays
    at full speed, and spreads activation-engine load.
  - exp is computed as 2^x (log2(e) folded into wq on host): half the
    tiles on the ACT engine (Exp with scale=ln2), half on the DVE via
    tensor_tensor(2, x, pow). Causal masking is a 0/1 bf16 multiply on
    GpSimd after exp (gpsimd cannot read PSUM, so it works on the SBUF
    probs, not the scores).
  - Attention output is normalized BEFORE the AllToAll (reciprocal of the
    ones-row denominator, broadcast via a rank-1 matmul), so the
    collective ships [256 feat, 256 tok] bf16 per destination and the
    o-projection starts immediately after the reshard.
  - One AllToAll reshards head-sharded -> sequence-sharded; each core then
    runs the o-projection (+ o-LoRA) for its 256 tokens; wo streams from
    HBM during phase D (bufs=2) instead of being cached in SBUF.

Numerics: bf16 operands, fp32 PSUM accumulation, fp32 softmax pieces.
RoPE layout: wq output features permuted on host so PSUM bank E holds all
four heads' even (real) dims and bank O the odd dims; RoPE is then plain
full-width [128,512] vector ops straight out of PSUM.
"""

import sys

for _p in ("/opt/trn_rl_repo", "/root/.axon_site/_ro/trn_rl_repo"):
    if _p not in sys.path:
        sys.path.insert(0, _p)

import numpy as np
import ml_dtypes

import concourse.bass as bass
import concourse.tile as tile
from concourse import bacc, mybir
from concourse.masks import make_identity
from concourse.alu_op_type import AluOpType

F32 = mybir.dt.float32
BF16 = mybir.dt.bfloat16
AF = mybir.ActivationFunctionType
AX = mybir.AxisListType
BF16NP = ml_dtypes.bfloat16

B, S, D = 1, 2048, 2048
H, KVH, HD = 32, 8, 64
NREP = H // KVH
R, E = 8, 8
SCALING = 32.0 / 8.0
NCORES = 8
QH = H // NCORES          # 4 q heads per core
QF = QH * HD              # 256 q feats per core
KF = HD                   # 64 kv feats per core
TSH = S // NCORES         # 256 tokens per core for o-proj
NKT = S // 128            # 16 key tiles
NQB = S // 512            # 4 query blocks
NIF = D // 128            # 16 contraction tiles

LN2 = float(np.log(2.0))
MASK_NEG = -1e30
M_SKIP, M_ZERO, M_ADD = 0, 1, 2




def _perm_eo():
    """Bank-E/bank-O feature permutations (within a core's 256 q feats)."""
    idx_e = np.zeros(128, dtype=np.int64)
    idx_o = np.zeros(128, dtype=np.int64)
    for p in range(128):
        h, j = p // 32, p % 32
        idx_e[p] = 64 * h + 2 * j
        idx_o[p] = 64 * h + 2 * j + 1
    return idx_e, idx_o


IDX_QE, IDX_QO = _perm_eo()
IDX_K = np.concatenate([2 * np.arange(32), 2 * np.arange(32) + 1])


def _a64(A):
    """[E,R,D] -> [D, 64] stationary with col r*8+e."""
    return np.transpose(A, (1, 0, 2)).reshape(E * R, -1).T


def _b_flat(Bw, scale):
    """[E, OF, R] -> [64, OF] with row r*8+e."""
    return np.transpose(Bw, (2, 0, 1)).reshape(E * R, -1) * scale


def _bf(x):
    return np.ascontiguousarray(x, dtype=np.float32).astype(BF16NP)


def _f32(x):
    return np.ascontiguousarray(x, dtype=np.float32)


def classify_mask(maskT):
    """maskT: [S(k), S(q)] clamped fp32. Returns [NKT, NQB] class map."""
    cls = np.zeros((NKT, NQB), dtype=np.int64)
    for kt in range(NKT):
        blk_rows = maskT[kt * 128:(kt + 1) * 128]
        for qb in range(NQB):
            blk = blk_rows[:, qb * 512:(qb + 1) * 512]
            if np.all(blk <= MASK_NEG * 0.5):
                cls[kt, qb] = M_SKIP
            elif np.all(blk == 0.0):
                cls[kt, qb] = M_ZERO
            else:
                cls[kt, qb] = M_ADD
    return cls


def build(mask_cls):
    nc = bacc.Bacc(None, target_bir_lowering=False)

    xT = nc.declare_dram_parameter("xT", [D, S], BF16, isOutput=False)
    wqT = nc.declare_dram_parameter("wqT", [D, 256], BF16, isOutput=False)
    wkv = nc.declare_dram_parameter("wkv", [D, 128], BF16, isOutput=False)
    l1 = nc.declare_dram_parameter("l1", [D, 128], BF16, isOutput=False)
    l2 = nc.declare_dram_parameter("l2", [D, 88], BF16, isOutput=False)
    ao = nc.declare_dram_parameter("ao", [D, 72], BF16, isOutput=False)
    bqe = nc.declare_dram_parameter("bqe", [64, 128], BF16, isOutput=False)
    bqo = nc.declare_dram_parameter("bqo", [64, 128], BF16, isOutput=False)
    bkv = nc.declare_dram_parameter("bkv", [128, 128], BF16, isOutput=False)
    bo = nc.declare_dram_parameter("bo", [64, D], BF16, isOutput=False)
    woT = nc.declare_dram_parameter("woT", [D, D], BF16, isOutput=False)
    cs = nc.declare_dram_parameter("cs", [128, S], BF16, isOutput=False)
    sn = nc.declare_dram_parameter("sn", [128, S], BF16, isOutput=False)
    m01 = nc.declare_dram_parameter("m01", [NQB * 4 * 128, 512], BF16,
                                    isOutput=False)
    y = nc.declare_dram_parameter("y", [TSH, D], F32, isOutput=True)

    cc_in = nc.dram_tensor("cc_in", [NCORES, QF, TSH], BF16)
    cc_out = nc.dram_tensor("cc_out", [NCORES, QF, TSH], BF16)

    with tile.TileContext(nc) as tc:
        _emit(nc, tc, locals(), mask_cls)
    nc.finalize()
    return nc


def _emit(nc, tc, t, mask_cls):
    xT, wqT, wkv, l1, l2, ao = (t["xT"], t["wqT"], t["wkv"], t["l1"],
                                t["l2"], t["ao"])
    bqe, bqo, bkv, bo, woT = t["bqe"], t["bqo"], t["bkv"], t["bo"], t["woT"]
    cs, sn, m01, y = t["cs"], t["sn"], t["m01"], t["y"]
    cc_in, cc_out = t["cc_in"], t["cc_out"]

    import contextlib
    ctx = contextlib.ExitStack()
    with ctx:
        pp = ctx.enter_context(tc.tile_pool(name="pp", bufs=1))
        ps = ctx.enter_context(tc.tile_pool(name="ps", bufs=1, space="PSUM"))
        pd = ctx.enter_context(tc.tile_pool(name="pdram", bufs=2,
                                            space="DRAM"))

        # ---- persistent weights ----
        l1_sb = pp.tile([128, NIF, 128], BF16)
        nc.sync.dma_start(out=l1_sb, in_=l1.rearrange("(n p) f -> p n f",
                                                      p=128))
        l2_sb = pp.tile([128, NIF, 88], BF16)
        nc.sync.dma_start(out=l2_sb, in_=l2.rearrange("(n p) f -> p n f",
                                                      p=128))
        wqT_sb = pp.tile([128, NIF, 256], BF16)
        nc.sync.dma_start(out=wqT_sb, in_=wqT.rearrange("(n p) f -> p n f",
                                                        p=128))
        wkv_sb = pp.tile([128, NIF, 128], BF16)
        nc.sync.dma_start(out=wkv_sb, in_=wkv.rearrange("(n p) f -> p n f",
                                                        p=128))
        bqe_sb = pp.tile([64, 128], BF16)
        nc.gpsimd.dma_start(out=bqe_sb, in_=bqe[:])
        bqo_sb = pp.tile([64, 128], BF16)
        nc.gpsimd.dma_start(out=bqo_sb, in_=bqo[:])
        bkv_sb = pp.tile([128, 128], BF16)
        nc.gpsimd.dma_start(out=bkv_sb, in_=bkv[:])
        ao_sb = pp.tile([128, NIF, 72], BF16)
        nc.gpsimd.dma_start(out=ao_sb, in_=ao.rearrange("(n p) f -> p n f",
                                                        p=128))
        bo_sb = pp.tile([64, D], BF16)
        nc.gpsimd.dma_start(out=bo_sb, in_=bo[:])

        ident_f = pp.tile([128, 128], F32)
        make_identity(nc, ident_f)
        ident_b = pp.tile([128, 128], BF16)
        make_identity(nc, ident_b)
        ones_f = pp.tile([1, 64], F32)
        nc.vector.memset(ones_f, 1.0)

        # persistent attention operands
        qh_sb = pp.tile([128, 2, S], BF16)     # head-contig rotated q
        kh_sb = pp.tile([128, S], BF16)        # kv head dup in both halves
        vtok = pp.tile([128, NKT, 65], BF16)   # token-major v + ones col
        nc.vector.memset(vtok, 0.0)
        for kt in range(NKT):
            nc.vector.memset(vtok[:, kt, 64:65], 1.0)
        g_sb = pp.tile([128, NIF, TSH], BF16)  # gathered out (post-A2A)

        # PSUM tags: o0-o3 (outp / proj q_e,q_o,kv,tp) s0-s3 (scores / L1,L2)
        OT = ["o0", "o1", "o2", "o3"]
        ST = ["s0", "s1", "s2", "s3"]

        def rw_chain(pool, lg_ps, ngrp, ntok, tag):
            """Batched router softmax.

            lg_ps: [8*ngrp, ntok] f32 logits view (PSUM, any base).
            Returns DRAM handle rw_dr [8*ngrp, ntok] f32 holding softmax
            weights; caller DMA-broadcasts rows into SBUF.
            """
            nch = ntok // 128
            nr = 8 * ngrp
            lgT = pool.tile([nr, ntok], F32, name="lgT", tag="lgT", bufs=2)
            nc.vector.tensor_copy(lgT, lg_ps)
            tp_ps = ps.tile([128, nch * nr], F32, name="tp_ps", tag=OT[3])
            for c in range(nch):
                nc.tensor.transpose(tp_ps[:, nr * c:nr * c + nr],
                                    lgT[:, 128 * c:128 * c + 128],
                                    ident_f[0:nr, 0:nr])
            lgtok = pool.tile([128, nch, ngrp, 8], F32, name="lgtok",
                              tag="lgtok", bufs=2)
            nc.vector.tensor_copy(
                lgtok, tp_ps.rearrange("p (n g e) -> p n g e", g=ngrp, e=8))
            mx = pool.tile([128, nch, ngrp], F32, name="mx", tag="mx", bufs=2)
            nc.vector.tensor_reduce(mx, lgtok, axis=AX.X, op=AluOpType.max)
            lgs = pool.tile([128, nch, ngrp, 8], F32, name="lgs", tag="lgs",
                            bufs=2)
            nc.vector.tensor_tensor(
                lgs, lgtok,
                mx.unsqueeze(3).broadcast_to([128, nch, ngrp, 8]),
                AluOpType.subtract)
            ex = pool.tile([128, nch, ngrp, 8], F32, name="ex", tag="ex",
                           bufs=2)
            nc.scalar.activation(ex, lgs, AF.Exp)
            sm = pool.tile([128, nch, ngrp], F32, name="sm", tag="sm", bufs=2)
            nc.vector.tensor_reduce(sm, ex, axis=AX.X, op=AluOpType.add)
            rc = pool.tile([128, nch, ngrp], F32, name="rc", tag="rc", bufs=2)
            nc.vector.reciprocal(rc, sm)
            rw = pool.tile([128, nch, ngrp, 8], F32, name="rw", tag="rw",
                           bufs=2)
            nc.vector.tensor_tensor(
                rw, ex, rc.unsqueeze(3).broadcast_to([128, nch, ngrp, 8]),
                AluOpType.mult)
            rwT_ps = ps.tile([nr, ntok], F32, name="rwT_ps", tag=OT[3])
            for c in range(nch):
                nc.tensor.transpose(rwT_ps[:, 128 * c:128 * c + 128],
                                    rw[:, c, :, :], ident_f[:, 0:128])
            rwT = pool.tile([nr, ntok], F32, name="rwT", tag="rwT", bufs=2)
            nc.vector.tensor_copy(rwT, rwT_ps)
            rw_dr = pd.tile([nr, ntok], F32, name="rw_dr", tag=tag, bufs=2)
            nc.scalar.dma_start(out=rw_dr, in_=rwT)
            return rw_dr

        def rw_bcast(pool, rw_dr, grp, ntok, out_base, name):
            """Broadcast rows of group `grp` (8 rows) to 64 partitions
            (row r*8+e), into partitions [out_base, out_base+64)."""
            rwx = pool.tile([out_base + 64, ntok], F32, name=name, tag=name,
                            bufs=2)
            nc.scalar.dma_start(
                out=rwx[out_base:out_base + 64, :],
                in_=bass.AP(tensor=rw_dr.tensor,
                            offset=rw_dr.offset + 8 * grp * ntok,
                            ap=[[0, R], [ntok, R], [1, ntok]]))
            return rwx

        # =================== main interleaved loop ===================
        pA = ctx.enter_context(tc.tile_pool(name="pA", bufs=1))
        pC = ctx.enter_context(tc.tile_pool(name="pC", bufs=1))

        for i in range(NQB):
            tsl = slice(i * 512, (i + 1) * 512)
            # ---------------- proj(i) ----------------
            xq = pA.tile([128, NIF, 512], BF16, name="xq", tag="xq", bufs=2)
            nc.scalar.dma_start(
                out=xq, in_=xT.rearrange("(n p) t -> p n t", p=128)[:, :, tsl])
            csl_t = pA.tile([128, 512], BF16, name="csl", tag="csl", bufs=2)
            nc.gpsimd.dma_start(out=csl_t, in_=cs[:, tsl])
            ssl_t = pA.tile([128, 512], BF16, name="ssl", tag="ssl", bufs=2)
            nc.gpsimd.dma_start(out=ssl_t, in_=sn[:, tsl])
            m01_sb = pA.tile([128, 4, 512], BF16, name="m01", tag="m01",
                             bufs=2)
            nc.gpsimd.dma_start(
                out=m01_sb,
                in_=m01.rearrange("(n p) f -> p n f", p=128)[:, 4 * i:4 * i + 4, :])

            L1_ps = ps.tile([128, 512], F32, name="L1", tag=ST[0])
            L2_ps = ps.tile([88, 512], F32, name="L2", tag=ST[1])
            for k in range(NIF):
                st, sp = k == 0, k == NIF - 1
                nc.tensor.matmul(L1_ps, l1_sb[:, k, :], xq[:, k, :],
                                 start=st, stop=sp)
                nc.tensor.matmul(L2_ps, l2_sb[:, k, :], xq[:, k, :],
                                 start=st, stop=sp)
            rw_dr = rw_chain(pA, L2_ps[64:88, :], 3, 512, "rwqkv")
            rwx_q = rw_bcast(pA, rw_dr, 0, 512, 0, "rwx_q")
            rwx_k = rw_bcast(pA, rw_dr, 1, 512, 64, "rwx_k")
            rwx_v = rw_bcast(pA, rw_dr, 2, 512, 0, "rwx_v")

            qe_ps = ps.tile([128, 512], F32, name="qe", tag=OT[0])
            qo_ps = ps.tile([128, 512], F32, name="qo", tag=OT[1])
            kv_ps = ps.tile([128, 512], F32, name="kv", tag=OT[2])
            for k in range(NIF):
                rhs = xq[:, k, :]
                st = k == 0
                nc.tensor.matmul(qe_ps, wqT_sb[:, k, 0:128], rhs,
                                 start=st, stop=False)
                nc.tensor.matmul(qo_ps, wqT_sb[:, k, 128:256], rhs,
                                 start=st, stop=False)
                nc.tensor.matmul(kv_ps, wkv_sb[:, k, :], rhs,
                                 start=st, stop=False)

            hp_q = pA.tile([64, 512], BF16, name="hp_q", tag="hp_q", bufs=2)
            nc.vector.tensor_tensor(hp_q, L1_ps[0:64, :], rwx_q,
                                    AluOpType.mult)
            hp_kv = pA.tile([128, 512], BF16, name="hp_kv", tag="hp_kv",
                            bufs=2)
            nc.vector.tensor_tensor(hp_kv[64:128, :], L1_ps[64:128, :],
                                    rwx_k[64:128, :], AluOpType.mult)
            nc.vector.tensor_tensor(hp_kv[0:64, :], L2_ps[0:64, :], rwx_v,
                                    AluOpType.mult)
            nc.tensor.matmul(qe_ps, bqe_sb, hp_q, start=False, stop=True)
            nc.tensor.matmul(qo_ps, bqo_sb, hp_q, start=False, stop=True)
            nc.tensor.matmul(kv_ps, bkv_sb, hp_kv, start=False, stop=True)

            # ---- RoPE: q straight out of the two PSUM banks ----
            csl = csl_t
            ssl = ssl_t
            tm1 = pA.tile([128, 512], F32, name="tm1", tag="tm1", bufs=2)
            tm2 = pA.tile([128, 512], F32, name="tm2", tag="tm2", bufs=2)
            qre = pA.tile([128, 512], BF16, name="qre", tag="qre", bufs=2)
            qro = pA.tile([128, 512], BF16, name="qro", tag="qro", bufs=2)
            nc.vector.tensor_tensor(tm1, qe_ps, csl, AluOpType.mult)
            nc.vector.tensor_tensor(tm2, qo_ps, ssl, AluOpType.mult)
            nc.vector.tensor_tensor(qre, tm1, tm2, AluOpType.subtract)
            nc.vector.tensor_tensor(tm1, qe_ps, ssl, AluOpType.mult)
            nc.vector.tensor_tensor(tm2, qo_ps, csl, AluOpType.mult)
            nc.vector.tensor_tensor(qro, tm1, tm2, AluOpType.add)
            for h in range(QH):
                page, half = h // 2, h % 2
                nc.scalar.dma_start(
                    out=qh_sb[64 * half:64 * half + 32, page, tsl],
                    in_=qre[32 * h:32 * h + 32, :])
                nc.scalar.dma_start(
                    out=qh_sb[64 * half + 32:64 * half + 64, page, tsl],
                    in_=qro[32 * h:32 * h + 32, :])

            # ---- k rope (32-row slabs) + v extraction ----
            kpre = pA.tile([32, 2, 512], F32, name="kpre", tag="kpre", bufs=2)
            nc.vector.tensor_copy(kpre[:, 0, :], kv_ps[0:32, :])
            nc.vector.tensor_copy(kpre[:, 1, :], kv_ps[32:64, :])
            krot = pA.tile([32, 2, 512], BF16, name="krot", tag="krot",
                           bufs=2)
            te = pA.tile([32, 512], F32, name="te", tag="te", bufs=2)
            to = pA.tile([32, 512], F32, name="to", tag="to", bufs=2)
            nc.vector.tensor_tensor(te, kpre[:, 0, :], csl[0:32, :],
                                    AluOpType.mult)
            nc.vector.tensor_tensor(to, kpre[:, 1, :], ssl[0:32, :],
                                    AluOpType.mult)
            nc.vector.tensor_tensor(krot[:, 0, :], te, to, AluOpType.subtract)
            nc.vector.tensor_tensor(te, kpre[:, 0, :], ssl[0:32, :],
                                    AluOpType.mult)
            nc.vector.tensor_tensor(to, kpre[:, 1, :], csl[0:32, :],
                                    AluOpType.mult)
            nc.vector.tensor_tensor(krot[:, 1, :], te, to, AluOpType.add)
            vT_sb = pA.tile([64, 512], BF16, name="vT", tag="vT", bufs=2)
            for half in range(2):
                nc.scalar.dma_start(out=kh_sb[64 * half:64 * half + 32, tsl],
                                    in_=krot[:, 0, :])
                nc.scalar.dma_start(
                    out=kh_sb[64 * half + 32:64 * half + 64, tsl],
                    in_=krot[:, 1, :])
            nc.vector.tensor_copy(vT_sb, kv_ps[64:128, :])
            for j in range(4):
                kt = 4 * i + j
                v_ps = ps.tile([128, 64], BF16, name="v_ps", tag=OT[3])
                nc.tensor.transpose(v_ps, vT_sb[:, 128 * j:128 * j + 128],
                                    ident_b[0:64, 0:64])
                nc.vector.tensor_copy(vtok[:, kt, 0:64], v_ps)

            # ---------------- attn(qb=i) ----------------
            qsl = tsl
            active = [kt for kt in range(NKT) if mask_cls[kt, i] != M_SKIP]
            assert active
            outps = [ps.tile([65, 512], F32, name="outp%d" % h, tag=OT[h])
                     for h in range(QH)]
            prev = None
            for n_kt, kt in enumerate(active):
                ksl = slice(128 * kt, 128 * kt + 128)
                madd = mask_cls[kt, i] == M_ADD
                # diagonal tile j: query columns < 128j are fully masked
                off = 128 * (kt - active[-4]) if madd else 0
                osl = slice(i * 512 + off, (i + 1) * 512)
                scs = []
                for h in range(QH):
                    page, half = h // 2, h % 2
                    sc = ps.tile([128, 512], F32, name="sc%d" % h,
                                 tag=ST[h])
                    nc.tensor.matmul(
                        sc[:, off:], kh_sb[64 * half:64 * half + 64, ksl],
                        qh_sb[64 * half:64 * half + 64, page, osl],
                        start=True, stop=True, tile_position=(64 * half, 0))
                    scs.append(sc)
                if prev is not None:
                    pkt, pprs, poff = prev
                    for h in range(QH):
                        nc.tensor.matmul(outps[h][:, poff:],
                                         vtok[:, pkt, :], pprs[h][:, poff:],
                                         start=(pkt == active[0]), stop=False)
                prs = []
                for h in range(QH):
                    pr = pC.tile([128, 512], BF16, name="pr", tag="pr",
                                 bufs=12)
                    nc.scalar.activation(pr[:, off:], scs[h][:, off:],
                                         AF.Exp, scale=LN2)
                    if madd:
                        mi = kt - active[-4]
                        nc.gpsimd.tensor_tensor(pr[:, off:], pr[:, off:],
                                                m01_sb[:, mi, off:],
                                                AluOpType.mult)
                    prs.append(pr)
                prev = (kt, prs, off)
            pkt, pprs, poff = prev
            for h in range(QH):
                nc.tensor.matmul(outps[h][:, poff:], vtok[:, pkt, :],
                                 pprs[h][:, poff:],
                                 start=(pkt == active[0]), stop=True)

            # ---- normalize + ship ----
            for h in range(QH):
                den_r = pC.tile([1, 512], F32, name="den_r", tag="den_r",
                                bufs=4)
                nc.vector.reciprocal(den_r, outps[h][64:65, :])
                rb_ps = ps.tile([64, 512], F32, name="rb", tag=ST[h])
                nc.tensor.matmul(rb_ps, ones_f, den_r, start=True, stop=True)
                rbc = pC.tile([64, 512], F32, name="rbc", tag="rbc", bufs=4)
                nc.vector.tensor_copy(rbc, rb_ps)
                on64 = pC.tile([64, 512], BF16, name="on64", tag="on64",
                               bufs=8)
                nc.vector.tensor_tensor(on64, outps[h][0:64, :], rbc,
                                        AluOpType.mult)
                for half in range(2):
                    nc.gpsimd.dma_start(
                        out=cc_in[2 * i + half, 64 * h:64 * h + 64, :],
                        in_=on64[:, 256 * half:256 * half + 256])

        # =================== A2A + o-proj ===================
        nc.gpsimd.collective_compute(
            "AllToAll", AluOpType.bypass, ins=[cc_in[:]], outs=[cc_out[:]],
            replica_groups=[list(range(NCORES))])

        g_v = g_sb.rearrange("p (c n) t -> p c n t", n=2)
        for n in range(2):
            nc.sync.dma_start(
                out=g_v[:, :, n, :],
                in_=cc_out[:, 128 * n:128 * n + 128, :]
                    .rearrange("c p t -> p c t"))

        ho_ps = ps.tile([72, TSH], F32, name="ho", tag=ST[0])
        for k in range(NIF):
            nc.tensor.matmul(ho_ps, ao_sb[:, k, :], g_sb[:, k, :],
                             start=(k == 0), stop=(k == NIF - 1))
        rwo_dr = rw_chain(pC, ho_ps[64:72, :], 1, TSH, "rwo")
        rwx_o = rw_bcast(pC, rwo_dr, 0, TSH, 0, "rwx_o")
        hpo = pC.tile([64, TSH], BF16, name="hpo")
        nc.vector.tensor_tensor(hpo, ho_ps[0:64, :], rwx_o, AluOpType.mult)

        for ob in range(4):
            osl = slice(ob * 512, (ob + 1) * 512)
            # reuse the xq slot (same shape, xq dead by phase D)
            wo_sb = pA.tile([128, NIF, 512], BF16, name="xq", tag="xq",
                            bufs=2)
            nc.sync.dma_start(
                out=wo_sb,
                in_=woT.rearrange("(n p) f -> p n f", p=128)[:, :, osl])
            for tt in range(2):
                yp = ps.tile([128, 512], F32, name="yp",
                             tag=OT[2 * (ob % 2) + tt])
                for k in range(NIF):
                    nc.tensor.matmul(yp, g_sb[:, k, 128 * tt:128 * tt + 128],
                                     wo_sb[:, k, :], start=(k == 0),
                                     stop=False)
                nc.tensor.matmul(yp, hpo[:, 128 * tt:128 * tt + 128],
                                 bo_sb[:, osl], start=False, stop=True)
                yt = pC.tile([128, 512], F32, name="yt", tag="yt", bufs=2)
                nc.vector.tensor_copy(yt, yp)
                nc.sync.dma_start(out=y[128 * tt:128 * tt + 128, osl],
                                  in_=yt)


# ======================= host side =======================

_CACHE = {}


def _prep_inputs(x, mask, freqs_cos, freqs_sin, wq, wk, wv, wo,
                 lq_router, lq_A, lq_B, lk_router, lk_A, lk_B,
                 lv_router, lv_A, lv_B, lo_router, lo_A, lo_B):
    scale = float(np.log2(np.e)) / np.sqrt(HD)  # log2e folded: exp via 2^x
    x = _f32(np.asarray(x)).reshape(S, D)
    maskf = _f32(np.asarray(mask)).reshape(S, S)
    maskT = np.maximum(maskf, MASK_NEG).T.copy()
    mask_cls = classify_mask(maskT)

    xTb = _bf(x.T)
    cs4 = _bf(np.tile(_f32(freqs_cos).T, (4, 1)))      # [128, S]
    sn4 = _bf(np.tile(_f32(freqs_sin).T, (4, 1)))
    woTb = _bf(_f32(wo).T)

    # 0/1 mask tiles for the diagonal (M_ADD) blocks, stacked [16*128, 512]
    m01 = np.zeros((NQB * 4 * 128, 512), dtype=np.float32)
    for qb in range(NQB):
        adds = [kt for kt in range(NKT) if mask_cls[kt, qb] == M_ADD]
        for j, kt in enumerate(adds[-4:]):
            blk = maskT[128 * kt:128 * kt + 128,
                        512 * qb:512 * qb + 512]
            m01[128 * (4 * qb + j):128 * (4 * qb + j + 1)] = (blk == 0.0)

    ao_p = _bf(np.concatenate([_a64(_f32(lo_A)), _f32(lo_router).T], axis=1))
    bo_f = _bf(_b_flat(_f32(lo_B), SCALING))

    shared = dict(xT=xTb, cs=cs4, sn=sn4, woT=woTb, m01=_bf(m01),
                  ao=ao_p, bo=bo_f)

    l1_p = _bf(np.concatenate([_a64(_f32(lq_A)), _a64(_f32(lk_A))], axis=1))
    l2_p = _bf(np.concatenate([_a64(_f32(lv_A)), _f32(lq_router).T,
                               _f32(lk_router).T, _f32(lv_router).T], axis=1))

    wqf, wkf, wvf = _f32(wq), _f32(wk), _f32(wv)
    lqB, lkB, lvB = _f32(lq_B), _f32(lk_B), _f32(lv_B)

    in_maps = []
    for c in range(NCORES):
        wq_c = wqf[c * QF:(c + 1) * QF] * scale
        wqT_c = np.concatenate([wq_c[IDX_QE].T, wq_c[IDX_QO].T], axis=1)
        wk_c = wkf[c * KF:(c + 1) * KF][IDX_K]
        wv_c = wvf[c * KF:(c + 1) * KF]
        wkv_c = np.concatenate([wk_c.T, wv_c.T], axis=1)
        bq_c = _b_flat(lqB[:, c * QF:(c + 1) * QF, :], SCALING * scale)
        bk_c = _b_flat(lkB[:, c * KF:(c + 1) * KF, :][:, IDX_K, :], SCALING)
        bv_c = _b_flat(lvB[:, c * KF:(c + 1) * KF, :], SCALING)
        # hp_kv rows 0:64 = h_v*rw_v, rows 64:128 = h_k*rw_k;
        # kv out rows 0:64 = k-proj, 64:128 = v-proj
        bkv_c = np.zeros((128, 128), dtype=np.float32)
        bkv_c[64:128, 0:64] = bk_c
        bkv_c[0:64, 64:128] = bv_c
        m = dict(shared)
        m.update(wqT=_bf(wqT_c), wkv=_bf(wkv_c), l1=l1_p, l2=l2_p,
                 bqe=_bf(bq_c[:, IDX_QE]), bqo=_bf(bq_c[:, IDX_QO]),
                 bkv=_bf(bkv_c))
        in_maps.append(m)
    return in_maps, mask_cls


def get_graph(mask_cls):
    key = mask_cls.tobytes()
    if key not in _CACHE:
        _CACHE[key] = build(mask_cls)
    return _CACHE[key]


def kernel(x, start_pos, mask, freqs_cos, freqs_sin, wq, wk, wv, wo,
           lq_router, lq_A, lq_B, lk_router, lk_A, lk_B,
           lv_router, lv_A, lv_B, lo_router, lo_A, lo_B,
           _trace=False):
    from concourse.bass_utils import run_bass_kernel_spmd
    in_maps, mask_cls = _prep_inputs(
        x, mask, freqs_cos, freqs_sin, wq, wk, wv, wo,
        lq_router, lq_A, lq_B, lk_router, lk_A, lk_B,
        lv_router, lv_A, lv_B, lo_router, lo_A, lo_B)
    nc = get_graph(mask_cls)
    res = run_bass_kernel_spmd(nc, in_maps, list(range(NCORES)), trace=_trace)
    out = np.concatenate([res.results[c]["y"] for c in range(NCORES)], axis=0)
    out = out.reshape(B, S, H * HD).astype(np.float32)
    if _trace:
        return out, res
    return out


# revision 27
# speedup vs baseline: 1.2066x; 1.2066x over previous
"""Trainium2 Bass kernel for MoE-LoRA GQA attention (nn_Attention_57389353009692).

Strategy (8 NeuronCores, one SPMD launch):
  - Tensor-parallel over heads: core c owns q-heads 4c..4c+3 and kv-head c.
  - Interleaved pipeline: for each 512-token block i: QKV projections
    (+MoE-LoRA, RoPE) for block i, then flash attention for query block i
    over key tiles 0..4i+3. Keeps the PE dense (projection matmuls fill
    the windows where attention waits on exp) so the HAM clock gate stays
    at full speed, and spreads activation-engine load.
  - exp is computed as 2^x (log2(e) folded into wq on host): half the
    tiles on the ACT engine (Exp with scale=ln2), half on the DVE via
    tensor_tensor(2, x, pow). Causal masking is a 0/1 bf16 multiply on
    GpSimd after exp (gpsimd cannot read PSUM, so it works on the SBUF
    probs, not the scores).
  - Attention output is normalized BEFORE the AllToAll (reciprocal of the
    ones-row denominator, broadcast via a rank-1 matmul), so the
    collective ships [256 feat, 256 tok] bf16 per destination and the
    o-projection starts immediately after the reshard.
  - One AllToAll reshards head-sharded -> sequence-sharded; each core then
    runs the o-projection (+ o-LoRA) for its 256 tokens; wo streams from
    HBM during phase D (bufs=2) instead of being cached in SBUF.

Numerics: bf16 operands, fp32 PSUM accumulation, fp32 softmax pieces.
RoPE layout: wq output features permuted on host so PSUM bank E holds all
four heads' even (real) dims and bank O the odd dims; RoPE is then plain
full-width [128,512] vector ops straight out of PSUM.
"""

import sys

for _p in ("/opt/trn_rl_repo", "/root/.axon_site/_ro/trn_rl_repo"):
    if _p not in sys.path:
        sys.path.insert(0, _p)

import numpy as np
import ml_dtypes

import concourse.bass as bass
import concourse.tile as tile
from concourse import bacc, mybir
from concourse.masks import make_identity
from concourse.alu_op_type import AluOpType

F32 = mybir.dt.float32
BF16 = mybir.dt.bfloat16
AF = mybir.ActivationFunctionType
AX = mybir.AxisListType
BF16NP = ml_dtypes.bfloat16

B, S, D = 1, 2048, 2048
H, KVH, HD = 32, 8, 64
NREP = H // KVH
R, E = 8, 8
SCALING = 32.0 / 8.0
NCORES = 8
QH = H // NCORES          # 4 q heads per core
QF = QH * HD              # 256 q feats per core
KF = HD                   # 64 kv feats per core
TSH = S // NCORES         # 256 tokens per core for o-proj
NKT = S // 128            # 16 key tiles
NQB = S // 512            # 4 query blocks
NIF = D // 128            # 16 contraction tiles

LN2 = float(np.log(2.0))
MASK_NEG = -1e30
M_SKIP, M_ZERO, M_ADD = 0, 1, 2




def _perm_eo():
    """Bank-E/bank-O feature permutations (within a core's 256 q feats)."""
    idx_e = np.zeros(128, dtype=np.int64)
    idx_o = np.zeros(128, dtype=np.int64)
    for p in range(128):
        h, j = p // 32, p % 32
        idx_e[p] = 64 * h + 2 * j
        idx_o[p] = 64 * h + 2 * j + 1
    return idx_e, idx_o


IDX_QE, IDX_QO = _perm_eo()
IDX_K = np.concatenate([2 * np.arange(32), 2 * np.arange(32) + 1])


def _a64(A):
    """[E,R,D] -> [D, 64] stationary with col r*8+e."""
    return np.transpose(A, (1, 0, 2)).reshape(E * R, -1).T


def _b_flat(Bw, scale):
    """[E, OF, R] -> [64, OF] with row r*8+e."""
    return np.transpose(Bw, (2, 0, 1)).reshape(E * R, -1) * scale


def _bf(x):
    return np.ascontiguousarray(x, dtype=np.float32).astype(BF16NP)


def _f32(x):
    return np.ascontiguousarray(x, dtype=np.float32)


def classify_mask(maskT):
    """maskT: [S(k), S(q)] clamped fp32. Returns [NKT, NQB] class map."""
    cls = np.zeros((NKT, NQB), dtype=np.int64)
    for kt in range(NKT):
        blk_rows = maskT[kt * 128:(kt + 1) * 128]
        for qb in range(NQB):
            blk = blk_rows[:, qb * 512:(qb + 1) * 512]
            if np.all(blk <= MASK_NEG * 0.5):
                cls[kt, qb] = M_SKIP
            elif np.all(blk == 0.0):
                cls[kt, qb] = M_ZERO
            else:
                cls[kt, qb] = M_ADD
    return cls


def build(mask_cls):
    nc = bacc.Bacc(None, target_bir_lowering=False)

    xT = nc.declare_dram_parameter("xT", [D, S], BF16, isOutput=False)
    wqT = nc.declare_dram_parameter("wqT", [D, 256], BF16, isOutput=False)
    wkv = nc.declare_dram_parameter("wkv", [D, 128], BF16, isOutput=False)
    l1 = nc.declare_dram_parameter("l1", [D, 128], BF16, isOutput=False)
    l2 = nc.declare_dram_parameter("l2", [D, 88], BF16, isOutput=False)
    ao = nc.declare_dram_parameter("ao", [D, 72], BF16, isOutput=False)
    bqe = nc.declare_dram_parameter("bqe", [64, 128], BF16, isOutput=False)
    bqo = nc.declare_dram_parameter("bqo", [64, 128], BF16, isOutput=False)
    bkv = nc.declare_dram_parameter("bkv", [128, 128], BF16, isOutput=False)
    bo = nc.declare_dram_parameter("bo", [64, D], BF16, isOutput=False)
    woT = nc.declare_dram_parameter("woT", [D, D], BF16, isOutput=False)
    cs = nc.declare_dram_parameter("cs", [128, S], BF16, isOutput=False)
    sn = nc.declare_dram_parameter("sn", [128, S], BF16, isOutput=False)
    m01 = nc.declare_dram_parameter("m01", [NQB * 4 * 128, 512], BF16,
                                    isOutput=False)
    y = nc.declare_dram_parameter("y", [TSH, D], F32, isOutput=True)

    sel = nc.declare_dram_parameter("sel", [H, NIF * 128], F32,
                                    isOutput=False)
    cc_in = nc.dram_tensor("cc_in", [NCORES, QF + QH, TSH], BF16)
    cc_out = nc.dram_tensor("cc_out", [NCORES, QF + QH, TSH], BF16)

    with tile.TileContext(nc) as tc:
        _emit(nc, tc, locals(), mask_cls)
    nc.finalize()
    return nc


def _emit(nc, tc, t, mask_cls):
    xT, wqT, wkv, l1, l2, ao = (t["xT"], t["wqT"], t["wkv"], t["l1"],
                                t["l2"], t["ao"])
    bqe, bqo, bkv, bo, woT = t["bqe"], t["bqo"], t["bkv"], t["bo"], t["woT"]
    cs, sn, m01, y, sel = t["cs"], t["sn"], t["m01"], t["y"], t["sel"]
    cc_in, cc_out = t["cc_in"], t["cc_out"]

    import contextlib
    ctx = contextlib.ExitStack()
    with ctx:
        pp = ctx.enter_context(tc.tile_pool(name="pp", bufs=1))
        ps = ctx.enter_context(tc.tile_pool(name="ps", bufs=1, space="PSUM"))
        pd = ctx.enter_context(tc.tile_pool(name="pdram", bufs=2,
                                            space="DRAM"))

        # ---- persistent weights ----
        l1_sb = pp.tile([128, NIF, 128], BF16)
        nc.sync.dma_start(out=l1_sb, in_=l1.rearrange("(n p) f -> p n f",
                                                      p=128))
        l2_sb = pp.tile([128, NIF, 88], BF16)
        nc.sync.dma_start(out=l2_sb, in_=l2.rearrange("(n p) f -> p n f",
                                                      p=128))
        wqT_sb = pp.tile([128, NIF, 256], BF16)
        nc.sync.dma_start(out=wqT_sb, in_=wqT.rearrange("(n p) f -> p n f",
                                                        p=128))
        wkv_sb = pp.tile([128, NIF, 128], BF16)
        nc.sync.dma_start(out=wkv_sb, in_=wkv.rearrange("(n p) f -> p n f",
                                                        p=128))
        bqe_sb = pp.tile([64, 128], BF16)
        nc.gpsimd.dma_start(out=bqe_sb, in_=bqe[:])
        bqo_sb = pp.tile([64, 128], BF16)
        nc.gpsimd.dma_start(out=bqo_sb, in_=bqo[:])
        bkv_sb = pp.tile([128, 128], BF16)
        nc.gpsimd.dma_start(out=bkv_sb, in_=bkv[:])
        ao_sb = pp.tile([128, NIF, 72], BF16)
        nc.gpsimd.dma_start(out=ao_sb, in_=ao.rearrange("(n p) f -> p n f",
                                                        p=128))
        bo_sb = pp.tile([64, D], BF16)
        nc.gpsimd.dma_start(out=bo_sb, in_=bo[:])
        sel_sb = pp.tile([H, NIF * 128], F32)
        nc.gpsimd.dma_start(out=sel_sb, in_=sel[:])

        ident_f = pp.tile([128, 128], F32)
        make_identity(nc, ident_f)
        ident_b = pp.tile([128, 128], BF16)
        make_identity(nc, ident_b)


        # persistent attention operands
        qh_sb = pp.tile([128, 2, S], BF16)     # head-contig rotated q
        kh_sb = pp.tile([128, S], BF16)        # kv head dup in both halves
        vtok = pp.tile([128, NKT, 65], BF16)   # token-major v + ones col
        nc.vector.memset(vtok, 0.0)
        for kt in range(NKT):
            nc.vector.memset(vtok[:, kt, 64:65], 1.0)
        g_sb = pp.tile([128, NIF, TSH], BF16)  # gathered out (post-A2A)

        # PSUM tags: o0-o3 (outp / proj q_e,q_o,kv,tp) s0-s3 (scores / L1,L2)
        OT = ["o0", "o1", "o2", "o3"]
        ST = ["s0", "s1", "s2", "s3"]

        def rw_chain(pool, lg_ps, ngrp, ntok, tag):
            """Batched router softmax.

            lg_ps: [8*ngrp, ntok] f32 logits view (PSUM, any base).
            Returns DRAM handle rw_dr [8*ngrp, ntok] f32 holding softmax
            weights; caller DMA-broadcasts rows into SBUF.
            """
            nch = ntok // 128
            nr = 8 * ngrp
            lgT = pool.tile([nr, ntok], F32, name="lgT", tag="lgT", bufs=2)
            nc.vector.tensor_copy(lgT, lg_ps)
            tp_ps = ps.tile([128, nch * nr], F32, name="tp_ps", tag=OT[3])
            for c in range(nch):
                nc.tensor.transpose(tp_ps[:, nr * c:nr * c + nr],
                                    lgT[:, 128 * c:128 * c + 128],
                                    ident_f[0:nr, 0:nr])
            lgtok = pool.tile([128, nch, ngrp, 8], F32, name="lgtok",
                              tag="lgtok", bufs=2)
            nc.vector.tensor_copy(
                lgtok, tp_ps.rearrange("p (n g e) -> p n g e", g=ngrp, e=8))
            mx = pool.tile([128, nch, ngrp], F32, name="mx", tag="mx", bufs=2)
            nc.vector.tensor_reduce(mx, lgtok, axis=AX.X, op=AluOpType.max)
            lgs = pool.tile([128, nch, ngrp, 8], F32, name="lgs", tag="lgs",
                            bufs=2)
            nc.vector.tensor_tensor(
                lgs, lgtok,
                mx.unsqueeze(3).broadcast_to([128, nch, ngrp, 8]),
                AluOpType.subtract)
            ex = pool.tile([128, nch, ngrp, 8], F32, name="ex", tag="ex",
                           bufs=2)
            nc.scalar.activation(ex, lgs, AF.Exp)
            sm = pool.tile([128, nch, ngrp], F32, name="sm", tag="sm", bufs=2)
            nc.vector.tensor_reduce(sm, ex, axis=AX.X, op=AluOpType.add)
            rc = pool.tile([128, nch, ngrp], F32, name="rc", tag="rc", bufs=2)
            nc.vector.reciprocal(rc, sm)
            rw = pool.tile([128, nch, ngrp, 8], F32, name="rw", tag="rw",
                           bufs=2)
            nc.vector.tensor_tensor(
                rw, ex, rc.unsqueeze(3).broadcast_to([128, nch, ngrp, 8]),
                AluOpType.mult)
            rwT_ps = ps.tile([nr, ntok], F32, name="rwT_ps", tag=OT[3])
            for c in range(nch):
                nc.tensor.transpose(rwT_ps[:, 128 * c:128 * c + 128],
                                    rw[:, c, :, :], ident_f[:, 0:128])
            rwT = pool.tile([nr, ntok], F32, name="rwT", tag="rwT", bufs=2)
            nc.vector.tensor_copy(rwT, rwT_ps)
            rw_dr = pd.tile([nr, ntok], F32, name="rw_dr", tag=tag, bufs=2)
            nc.scalar.dma_start(out=rw_dr, in_=rwT)
            return rw_dr

        def rw_bcast(pool, rw_dr, grp, ntok, out_base, name):
            """Broadcast rows of group `grp` (8 rows) to 64 partitions
            (row r*8+e), into partitions [out_base, out_base+64)."""
            rwx = pool.tile([out_base + 64, ntok], F32, name=name, tag=name,
                            bufs=2)
            nc.scalar.dma_start(
                out=rwx[out_base:out_base + 64, :],
                in_=bass.AP(tensor=rw_dr.tensor,
                            offset=rw_dr.offset + 8 * grp * ntok,
                            ap=[[0, R], [ntok, R], [1, ntok]]))
            return rwx

        # =================== main interleaved loop ===================
        pA = ctx.enter_context(tc.tile_pool(name="pA", bufs=1))
        pC = ctx.enter_context(tc.tile_pool(name="pC", bufs=1))

        for i in range(NQB):
            tsl = slice(i * 512, (i + 1) * 512)
            # ---------------- proj(i) ----------------
            xq = pA.tile([128, NIF, 512], BF16, name="xq", tag="xq", bufs=2)
            nc.scalar.dma_start(
                out=xq, in_=xT.rearrange("(n p) t -> p n t", p=128)[:, :, tsl])
            csl_t = pA.tile([128, 512], BF16, name="csl", tag="csl", bufs=2)
            nc.gpsimd.dma_start(out=csl_t, in_=cs[:, tsl])
            ssl_t = pA.tile([128, 512], BF16, name="ssl", tag="ssl", bufs=2)
            nc.gpsimd.dma_start(out=ssl_t, in_=sn[:, tsl])
            m01_sb = pA.tile([128, 4, 512], BF16, name="m01", tag="m01",
                             bufs=2)
            nc.gpsimd.dma_start(
                out=m01_sb,
                in_=m01.rearrange("(n p) f -> p n f", p=128)[:, 4 * i:4 * i + 4, :])

            L1_ps = ps.tile([128, 512], F32, name="L1", tag=ST[0])
            L2_ps = ps.tile([88, 512], F32, name="L2", tag=ST[1])
            for k in range(NIF):
                st, sp = k == 0, k == NIF - 1
                nc.tensor.matmul(L1_ps, l1_sb[:, k, :], xq[:, k, :],
                                 start=st, stop=sp)
                nc.tensor.matmul(L2_ps, l2_sb[:, k, :], xq[:, k, :],
                                 start=st, stop=sp)
            rw_dr = rw_chain(pA, L2_ps[64:88, :], 3, 512, "rwqkv")
            rwx_q = rw_bcast(pA, rw_dr, 0, 512, 0, "rwx_q")
            rwx_k = rw_bcast(pA, rw_dr, 1, 512, 64, "rwx_k")
            rwx_v = rw_bcast(pA, rw_dr, 2, 512, 0, "rwx_v")

            qe_ps = ps.tile([128, 512], F32, name="qe", tag=OT[0])
            qo_ps = ps.tile([128, 512], F32, name="qo", tag=OT[1])
            kv_ps = ps.tile([128, 512], F32, name="kv", tag=OT[2])
            for k in range(NIF):
                rhs = xq[:, k, :]
                st = k == 0
                nc.tensor.matmul(qe_ps, wqT_sb[:, k, 0:128], rhs,
                                 start=st, stop=False)
                nc.tensor.matmul(qo_ps, wqT_sb[:, k, 128:256], rhs,
                                 start=st, stop=False)
                nc.tensor.matmul(kv_ps, wkv_sb[:, k, :], rhs,
                                 start=st, stop=False)

            hp_q = pA.tile([64, 512], BF16, name="hp_q", tag="hp_q", bufs=2)
            nc.vector.tensor_tensor(hp_q, L1_ps[0:64, :], rwx_q,
                                    AluOpType.mult)
            hp_kv = pA.tile([128, 512], BF16, name="hp_kv", tag="hp_kv",
                            bufs=2)
            nc.vector.tensor_tensor(hp_kv[64:128, :], L1_ps[64:128, :],
                                    rwx_k[64:128, :], AluOpType.mult)
            nc.vector.tensor_tensor(hp_kv[0:64, :], L2_ps[0:64, :], rwx_v,
                                    AluOpType.mult)
            nc.tensor.matmul(qe_ps, bqe_sb, hp_q, start=False, stop=True)
            nc.tensor.matmul(qo_ps, bqo_sb, hp_q, start=False, stop=True)
            nc.tensor.matmul(kv_ps, bkv_sb, hp_kv, start=False, stop=True)

            # ---- RoPE: q straight out of the two PSUM banks ----
            csl = csl_t
            ssl = ssl_t
            tm1 = pA.tile([128, 512], F32, name="tm1", tag="tm1", bufs=2)
            tm2 = pA.tile([128, 512], F32, name="tm2", tag="tm2", bufs=2)
            qre = pA.tile([128, 512], BF16, name="qre", tag="qre", bufs=2)
            qro = pA.tile([128, 512], BF16, name="qro", tag="qro", bufs=2)
            nc.vector.tensor_tensor(tm1, qe_ps, csl, AluOpType.mult)
            nc.vector.tensor_tensor(tm2, qo_ps, ssl, AluOpType.mult)
            nc.vector.tensor_tensor(qre, tm1, tm2, AluOpType.subtract)
            nc.vector.tensor_tensor(tm1, qe_ps, ssl, AluOpType.mult)
            nc.vector.tensor_tensor(tm2, qo_ps, csl, AluOpType.mult)
            nc.vector.tensor_tensor(qro, tm1, tm2, AluOpType.add)
            for h in range(QH):
                page, half = h // 2, h % 2
                nc.scalar.dma_start(
                    out=qh_sb[64 * half:64 * half + 32, page, tsl],
                    in_=qre[32 * h:32 * h + 32, :])
                nc.scalar.dma_start(
                    out=qh_sb[64 * half + 32:64 * half + 64, page, tsl],
                    in_=qro[32 * h:32 * h + 32, :])

            # ---- k rope (32-row slabs) + v extraction ----
            kpre = pA.tile([32, 2, 512], F32, name="kpre", tag="kpre", bufs=2)
            nc.vector.tensor_copy(kpre[:, 0, :], kv_ps[0:32, :])
            nc.vector.tensor_copy(kpre[:, 1, :], kv_ps[32:64, :])
            krot = pA.tile([32, 2, 512], BF16, name="krot", tag="krot",
                           bufs=2)
            te = pA.tile([32, 512], F32, name="te", tag="te", bufs=2)
            to = pA.tile([32, 512], F32, name="to", tag="to", bufs=2)
            nc.vector.tensor_tensor(te, kpre[:, 0, :], csl[0:32, :],
                                    AluOpType.mult)
            nc.vector.tensor_tensor(to, kpre[:, 1, :], ssl[0:32, :],
                                    AluOpType.mult)
            nc.vector.tensor_tensor(krot[:, 0, :], te, to, AluOpType.subtract)
            nc.vector.tensor_tensor(te, kpre[:, 0, :], ssl[0:32, :],
                                    AluOpType.mult)
            nc.vector.tensor_tensor(to, kpre[:, 1, :], csl[0:32, :],
                                    AluOpType.mult)
            nc.vector.tensor_tensor(krot[:, 1, :], te, to, AluOpType.add)
            vT_sb = pA.tile([64, 512], BF16, name="vT", tag="vT", bufs=2)
            for half in range(2):
                nc.scalar.dma_start(out=kh_sb[64 * half:64 * half + 32, tsl],
                                    in_=krot[:, 0, :])
                nc.scalar.dma_start(
                    out=kh_sb[64 * half + 32:64 * half + 64, tsl],
                    in_=krot[:, 1, :])
            nc.vector.tensor_copy(vT_sb, kv_ps[64:128, :])
            for j in range(4):
                kt = 4 * i + j
                v_ps = ps.tile([128, 64], BF16, name="v_ps", tag=OT[3])
                nc.tensor.transpose(v_ps, vT_sb[:, 128 * j:128 * j + 128],
                                    ident_b[0:64, 0:64])
                nc.vector.tensor_copy(vtok[:, kt, 0:64], v_ps)

            # ---------------- attn(qb=i) ----------------
            qsl = tsl
            active = [kt for kt in range(NKT) if mask_cls[kt, i] != M_SKIP]
            assert active
            outps = [ps.tile([65, 512], F32, name="outp%d" % h, tag=OT[h])
                     for h in range(QH)]
            prev = None
            for n_kt, kt in enumerate(active):
                ksl = slice(128 * kt, 128 * kt + 128)
                madd = mask_cls[kt, i] == M_ADD
                # diagonal tile j: query columns < 128j are fully masked
                off = 128 * (kt - active[-4]) if madd else 0
                osl = slice(i * 512 + off, (i + 1) * 512)
                scs = []
                for h in range(QH):
                    page, half = h // 2, h % 2
                    sc = ps.tile([128, 512], F32, name="sc%d" % h,
                                 tag=ST[h])
                    nc.tensor.matmul(
                        sc[:, off:], kh_sb[64 * half:64 * half + 64, ksl],
                        qh_sb[64 * half:64 * half + 64, page, osl],
                        start=True, stop=True, tile_position=(64 * half, 0))
                    scs.append(sc)
                if prev is not None:
                    pkt, pprs, poff = prev
                    for h in range(QH):
                        nc.tensor.matmul(outps[h][:, poff:],
                                         vtok[:, pkt, :], pprs[h][:, poff:],
                                         start=(pkt == active[0]), stop=False)
                prs = []
                for h in range(QH):
                    pr = pC.tile([128, 512], BF16, name="pr", tag="pr",
                                 bufs=12)
                    nc.scalar.activation(pr[:, off:], scs[h][:, off:],
                                         AF.Exp, scale=LN2)
                    if madd:
                        mi = kt - active[-4]
                        nc.vector.tensor_tensor(pr[:, off:], pr[:, off:],
                                                m01_sb[:, mi, off:],
                                                AluOpType.mult)
                    prs.append(pr)
                prev = (kt, prs, off)
            pkt, pprs, poff = prev
            for h in range(QH):
                nc.tensor.matmul(outps[h][:, poff:], vtok[:, pkt, :],
                                 pprs[h][:, poff:],
                                 start=(pkt == active[0]), stop=True)

            # ---- ship unnormalized sums + denominators ----
            for h in range(QH):
                on65 = pC.tile([65, 512], BF16, name="on65", tag="on65",
                               bufs=8)
                nc.vector.tensor_copy(on65, outps[h])
                for half in range(2):
                    hsl = slice(256 * half, 256 * half + 256)
                    nc.sync.dma_start(
                        out=cc_in[2 * i + half, 64 * h:64 * h + 64, :],
                        in_=on65[0:64, hsl])
                    nc.sync.dma_start(
                        out=cc_in[2 * i + half, QF + h, :],
                        in_=on65[64:65, hsl])

        # =================== A2A + o-proj ===================
        nc.gpsimd.collective_compute(
            "AllToAll", AluOpType.bypass, ins=[cc_in[:]], outs=[cc_out[:]],
            replica_groups=[list(range(NCORES))])

        g_v = g_sb.rearrange("p (c n) t -> p c n t", n=2)
        for n in range(2):
            nc.sync.dma_start(
                out=g_v[:, :, n, :],
                in_=cc_out[:, 128 * n:128 * n + 128, :]
                    .rearrange("c p t -> p c t"))
        den_all = pC.tile([32, TSH], BF16, name="den_all")
        for cb in range(NCORES):
            nc.sync.dma_start(out=den_all[QH * cb:QH * cb + QH, :],
                              in_=cc_out[cb, QF:QF + QH, :])
        rec32 = pC.tile([32, TSH], F32, name="rec32")
        nc.vector.reciprocal(rec32, den_all)
        g_n = pC.tile([128, NIF, TSH], BF16, name="g_n")
        for k in range(NIF):
            rb_ps = ps.tile([128, TSH], F32, name="rb_ps",
                            tag=ST[k % 4])
            nc.tensor.matmul(rb_ps, sel_sb[:, 128 * k:128 * k + 128],
                             rec32, start=True, stop=True)
            nc.vector.tensor_tensor(g_n[:, k, :], g_sb[:, k, :], rb_ps,
                                    AluOpType.mult)

        ho_ps = ps.tile([72, TSH], F32, name="ho", tag=ST[0])
        for k in range(NIF):
            nc.tensor.matmul(ho_ps, ao_sb[:, k, :], g_n[:, k, :],
                             start=(k == 0), stop=(k == NIF - 1))
        rwo_dr = rw_chain(pC, ho_ps[64:72, :], 1, TSH, "rwo")
        rwx_o = rw_bcast(pC, rwo_dr, 0, TSH, 0, "rwx_o")
        hpo = pC.tile([64, TSH], BF16, name="hpo")
        nc.vector.tensor_tensor(hpo, ho_ps[0:64, :], rwx_o, AluOpType.mult)

        for ob in range(4):
            osl = slice(ob * 512, (ob + 1) * 512)
            # reuse the xq slot (same shape, xq dead by phase D)
            wo_sb = pA.tile([128, NIF, 512], BF16, name="xq", tag="xq",
                            bufs=2)
            nc.sync.dma_start(
                out=wo_sb,
                in_=woT.rearrange("(n p) f -> p n f", p=128)[:, :, osl])
            for tt in range(2):
                yp = ps.tile([128, 512], F32, name="yp",
                             tag=OT[2 * (ob % 2) + tt])
                for k in range(NIF):
                    nc.tensor.matmul(yp, g_n[:, k, 128 * tt:128 * tt + 128],
                                     wo_sb[:, k, :], start=(k == 0),
                                     stop=False)
                nc.tensor.matmul(yp, hpo[:, 128 * tt:128 * tt + 128],
                                 bo_sb[:, osl], start=False, stop=True)
                yt = pC.tile([128, 512], F32, name="yt", tag="yt", bufs=2)
                nc.vector.tensor_copy(yt, yp)
                nc.sync.dma_start(out=y[128 * tt:128 * tt + 128, osl],
                                  in_=yt)


# ======================= host side =======================

_CACHE = {}


def _prep_inputs(x, mask, freqs_cos, freqs_sin, wq, wk, wv, wo,
                 lq_router, lq_A, lq_B, lk_router, lk_A, lk_B,
                 lv_router, lv_A, lv_B, lo_router, lo_A, lo_B):
    scale = float(np.log2(np.e)) / np.sqrt(HD)  # log2e folded: exp via 2^x
    x = _f32(np.asarray(x)).reshape(S, D)
    maskf = _f32(np.asarray(mask)).reshape(S, S)
    maskT = np.maximum(maskf, MASK_NEG).T.copy()
    mask_cls = classify_mask(maskT)

    xTb = _bf(x.T)
    cs4 = _bf(np.tile(_f32(freqs_cos).T, (4, 1)))      # [128, S]
    sn4 = _bf(np.tile(_f32(freqs_sin).T, (4, 1)))
    woTb = _bf(_f32(wo).T)

    # 0/1 mask tiles for the diagonal (M_ADD) blocks, stacked [16*128, 512]
    m01 = np.zeros((NQB * 4 * 128, 512), dtype=np.float32)
    for qb in range(NQB):
        adds = [kt for kt in range(NKT) if mask_cls[kt, qb] == M_ADD]
        for j, kt in enumerate(adds[-4:]):
            blk = maskT[128 * kt:128 * kt + 128,
                        512 * qb:512 * qb + 512]
            m01[128 * (4 * qb + j):128 * (4 * qb + j + 1)] = (blk == 0.0)

    ao_p = _bf(np.concatenate([_a64(_f32(lo_A)), _f32(lo_router).T], axis=1))
    bo_f = _bf(_b_flat(_f32(lo_B), SCALING))

    sel_m = np.zeros((H, NIF * 128), dtype=np.float32)
    for k in range(NIF):
        for p in range(128):
            sel_m[2 * k + p // 64, 128 * k + p] = 1.0
    shared = dict(xT=xTb, cs=cs4, sn=sn4, woT=woTb, m01=_bf(m01),
                  ao=ao_p, bo=bo_f, sel=sel_m)

    l1_p = _bf(np.concatenate([_a64(_f32(lq_A)), _a64(_f32(lk_A))], axis=1))
    l2_p = _bf(np.concatenate([_a64(_f32(lv_A)), _f32(lq_router).T,
                               _f32(lk_router).T, _f32(lv_router).T], axis=1))

    wqf, wkf, wvf = _f32(wq), _f32(wk), _f32(wv)
    lqB, lkB, lvB = _f32(lq_B), _f32(lk_B), _f32(lv_B)

    in_maps = []
    for c in range(NCORES):
        wq_c = wqf[c * QF:(c + 1) * QF] * scale
        wqT_c = np.concatenate([wq_c[IDX_QE].T, wq_c[IDX_QO].T], axis=1)
        wk_c = wkf[c * KF:(c + 1) * KF][IDX_K]
        wv_c = wvf[c * KF:(c + 1) * KF]
        wkv_c = np.concatenate([wk_c.T, wv_c.T], axis=1)
        bq_c = _b_flat(lqB[:, c * QF:(c + 1) * QF, :], SCALING * scale)
        bk_c = _b_flat(lkB[:, c * KF:(c + 1) * KF, :][:, IDX_K, :], SCALING)
        bv_c = _b_flat(lvB[:, c * KF:(c + 1) * KF, :], SCALING)
        # hp_kv rows 0:64 = h_v*rw_v, rows 64:128 = h_k*rw_k;
        # kv out rows 0:64 = k-proj, 64:128 = v-proj
        bkv_c = np.zeros((128, 128), dtype=np.float32)
        bkv_c[64:128, 0:64] = bk_c
        bkv_c[0:64, 64:128] = bv_c
        m = dict(shared)
        m.update(wqT=_bf(wqT_c), wkv=_bf(wkv_c), l1=l1_p, l2=l2_p,
                 bqe=_bf(bq_c[:, IDX_QE]), bqo=_bf(bq_c[:, IDX_QO]),
                 bkv=_bf(bkv_c))
        in_maps.append(m)
    return in_maps, mask_cls


def get_graph(mask_cls):
    key = mask_cls.tobytes()
    if key not in _CACHE:
        _CACHE[key] = build(mask_cls)
    return _CACHE[key]


def kernel(x, start_pos, mask, freqs_cos, freqs_sin, wq, wk, wv, wo,
           lq_router, lq_A, lq_B, lk_router, lk_A, lk_B,
           lv_router, lv_A, lv_B, lo_router, lo_A, lo_B,
           _trace=False):
    from concourse.bass_utils import run_bass_kernel_spmd
    in_maps, mask_cls = _prep_inputs(
        x, mask, freqs_cos, freqs_sin, wq, wk, wv, wo,
        lq_router, lq_A, lq_B, lk_router, lk_A, lk_B,
        lv_router, lv_A, lv_B, lo_router, lo_A, lo_B)
    nc = get_graph(mask_cls)
    res = run_bass_kernel_spmd(nc, in_maps, list(range(NCORES)), trace=_trace)
    out = np.concatenate([res.results[c]["y"] for c in range(NCORES)], axis=0)
    out = out.reshape(B, S, H * HD).astype(np.float32)
    if _trace:
        return out, res
    return out


# revision 36
# speedup vs baseline: 1.3078x; 1.0838x over previous
"""Trainium2 Bass kernel for MoE-LoRA GQA attention (nn_Attention_57389353009692).

Strategy (8 NeuronCores, one SPMD launch):
  - Tensor-parallel over heads: core c owns q-heads 4c..4c+3 and kv-head c.
  - Interleaved pipeline: for each 512-token block i: QKV projections
    (+MoE-LoRA, RoPE) for block i, then flash attention for query block i
    over key tiles 0..4i+3. Keeps the PE dense (projection matmuls fill
    the windows where attention waits on exp) so the HAM clock gate stays
    at full speed, and spreads activation-engine load.
  - exp is computed as 2^x (log2(e) folded into wq on host): half the
    tiles on the ACT engine (Exp with scale=ln2), half on the DVE via
    tensor_tensor(2, x, pow). Causal masking is a 0/1 bf16 multiply on
    GpSimd after exp (gpsimd cannot read PSUM, so it works on the SBUF
    probs, not the scores).
  - Attention output is normalized BEFORE the AllToAll (reciprocal of the
    ones-row denominator, broadcast via a rank-1 matmul), so the
    collective ships [256 feat, 256 tok] bf16 per destination and the
    o-projection starts immediately after the reshard.
  - One AllToAll reshards head-sharded -> sequence-sharded; each core then
    runs the o-projection (+ o-LoRA) for its 256 tokens; wo streams from
    HBM during phase D (bufs=2) instead of being cached in SBUF.

Numerics: bf16 operands, fp32 PSUM accumulation, fp32 softmax pieces.
RoPE layout: wq output features permuted on host so PSUM bank E holds all
four heads' even (real) dims and bank O the odd dims; RoPE is then plain
full-width [128,512] vector ops straight out of PSUM.
"""

import sys

for _p in ("/opt/trn_rl_repo", "/root/.axon_site/_ro/trn_rl_repo"):
    if _p not in sys.path:
        sys.path.insert(0, _p)

import numpy as np
import ml_dtypes

import concourse.bass as bass
import concourse.tile as tile
from concourse import bacc, mybir
from concourse.masks import make_identity
from concourse.alu_op_type import AluOpType

F32 = mybir.dt.float32
BF16 = mybir.dt.bfloat16
AF = mybir.ActivationFunctionType
AX = mybir.AxisListType
BF16NP = ml_dtypes.bfloat16

B, S, D = 1, 2048, 2048
H, KVH, HD = 32, 8, 64
NREP = H // KVH
R, E = 8, 8
SCALING = 32.0 / 8.0
NCORES = 8
QH = H // NCORES          # 4 q heads per core
QF = QH * HD              # 256 q feats per core
KF = HD                   # 64 kv feats per core
TSH = S // NCORES         # 256 tokens per core for o-proj
NKT = S // 128            # 16 key tiles
NQB = S // 512            # 4 query blocks
NIF = D // 128            # 16 contraction tiles

LN2 = float(np.log(2.0))
MASK_NEG = -1e30
M_SKIP, M_ZERO, M_ADD = 0, 1, 2




def _perm_eo():
    """Bank-E/bank-O feature permutations (within a core's 256 q feats)."""
    idx_e = np.zeros(128, dtype=np.int64)
    idx_o = np.zeros(128, dtype=np.int64)
    for p in range(128):
        h, j = p // 32, p % 32
        idx_e[p] = 64 * h + 2 * j
        idx_o[p] = 64 * h + 2 * j + 1
    return idx_e, idx_o


IDX_QE, IDX_QO = _perm_eo()
IDX_K = np.concatenate([2 * np.arange(32), 2 * np.arange(32) + 1])


def _a64(A):
    """[E,R,D] -> [D, 64] stationary with col r*8+e."""
    return np.transpose(A, (1, 0, 2)).reshape(E * R, -1).T


def _b_flat(Bw, scale):
    """[E, OF, R] -> [64, OF] with row r*8+e."""
    return np.transpose(Bw, (2, 0, 1)).reshape(E * R, -1) * scale


def _bf(x):
    return np.ascontiguousarray(x, dtype=np.float32).astype(BF16NP)


def _f32(x):
    return np.ascontiguousarray(x, dtype=np.float32)


def classify_mask(maskT):
    """maskT: [S(k), S(q)] clamped fp32. Returns [NKT, NQB] class map."""
    cls = np.zeros((NKT, NQB), dtype=np.int64)
    for kt in range(NKT):
        blk_rows = maskT[kt * 128:(kt + 1) * 128]
        for qb in range(NQB):
            blk = blk_rows[:, qb * 512:(qb + 1) * 512]
            if np.all(blk <= MASK_NEG * 0.5):
                cls[kt, qb] = M_SKIP
            elif np.all(blk == 0.0):
                cls[kt, qb] = M_ZERO
            else:
                cls[kt, qb] = M_ADD
    return cls


def build(mask_cls):
    nc = bacc.Bacc(None, target_bir_lowering=False)

    xT = nc.declare_dram_parameter("xT", [D, S], BF16, isOutput=False)
    wqT = nc.declare_dram_parameter("wqT", [D, 256], BF16, isOutput=False)
    wkv = nc.declare_dram_parameter("wkv", [D, 128], BF16, isOutput=False)
    l1 = nc.declare_dram_parameter("l1", [D, 128], BF16, isOutput=False)
    l2 = nc.declare_dram_parameter("l2", [D, 88], BF16, isOutput=False)
    ao = nc.declare_dram_parameter("ao", [D, 72], BF16, isOutput=False)
    bqe = nc.declare_dram_parameter("bqe", [64, 128], BF16, isOutput=False)
    bqo = nc.declare_dram_parameter("bqo", [64, 128], BF16, isOutput=False)
    bkv = nc.declare_dram_parameter("bkv", [128, 128], BF16, isOutput=False)
    bo = nc.declare_dram_parameter("bo", [64, D], BF16, isOutput=False)
    woT = nc.declare_dram_parameter("woT", [D, D], BF16, isOutput=False)
    cs = nc.declare_dram_parameter("cs", [128, S], BF16, isOutput=False)
    sn = nc.declare_dram_parameter("sn", [128, S], BF16, isOutput=False)
    m01 = nc.declare_dram_parameter("m01", [NQB * 4 * 128, 512], BF16,
                                    isOutput=False)
    y = nc.declare_dram_parameter("y", [TSH, D], F32, isOutput=True)

    sel = nc.declare_dram_parameter("sel", [H, NIF * 128], F32,
                                    isOutput=False)
    cc_in = nc.dram_tensor("cc_in", [NCORES, QF + QH, TSH], BF16)
    cc_out = nc.dram_tensor("cc_out", [NCORES, QF + QH, TSH], BF16)

    with tile.TileContext(nc) as tc:
        _emit(nc, tc, locals(), mask_cls)
    nc.finalize()
    return nc


def _emit(nc, tc, t, mask_cls):
    xT, wqT, wkv, l1, l2, ao = (t["xT"], t["wqT"], t["wkv"], t["l1"],
                                t["l2"], t["ao"])
    bqe, bqo, bkv, bo, woT = t["bqe"], t["bqo"], t["bkv"], t["bo"], t["woT"]
    cs, sn, m01, y, sel = t["cs"], t["sn"], t["m01"], t["y"], t["sel"]
    cc_in, cc_out = t["cc_in"], t["cc_out"]

    import contextlib
    ctx = contextlib.ExitStack()
    with ctx:
        pp = ctx.enter_context(tc.tile_pool(name="pp", bufs=1))
        ps = ctx.enter_context(tc.tile_pool(name="ps", bufs=1, space="PSUM"))
        pd = ctx.enter_context(tc.tile_pool(name="pdram", bufs=2,
                                            space="DRAM"))

        # ---- persistent weights ----
        l1_sb = pp.tile([128, NIF, 128], BF16)
        nc.sync.dma_start(out=l1_sb, in_=l1.rearrange("(n p) f -> p n f",
                                                      p=128))
        l2_sb = pp.tile([128, NIF, 88], BF16)
        nc.sync.dma_start(out=l2_sb, in_=l2.rearrange("(n p) f -> p n f",
                                                      p=128))
        wqT_sb = pp.tile([128, NIF, 256], BF16)
        nc.sync.dma_start(out=wqT_sb, in_=wqT.rearrange("(n p) f -> p n f",
                                                        p=128))
        wkv_sb = pp.tile([128, NIF, 128], BF16)
        nc.sync.dma_start(out=wkv_sb, in_=wkv.rearrange("(n p) f -> p n f",
                                                        p=128))
        bqe_sb = pp.tile([64, 128], BF16)
        nc.gpsimd.dma_start(out=bqe_sb, in_=bqe[:])
        bqo_sb = pp.tile([64, 128], BF16)
        nc.gpsimd.dma_start(out=bqo_sb, in_=bqo[:])
        bkv_sb = pp.tile([128, 128], BF16)
        nc.gpsimd.dma_start(out=bkv_sb, in_=bkv[:])
        ao_sb = pp.tile([128, NIF, 72], BF16)
        nc.gpsimd.dma_start(out=ao_sb, in_=ao.rearrange("(n p) f -> p n f",
                                                        p=128))
        bo_sb = pp.tile([64, D], BF16)
        nc.gpsimd.dma_start(out=bo_sb, in_=bo[:])
        sel_sb = pp.tile([H, NIF * 128], F32)
        nc.gpsimd.dma_start(out=sel_sb, in_=sel[:])

        ident_f = pp.tile([128, 128], F32)
        make_identity(nc, ident_f)
        ident_b = pp.tile([128, 128], BF16)
        make_identity(nc, ident_b)


        # persistent attention operands
        qh_sb = pp.tile([128, 2, S], BF16)     # head-contig rotated q
        kh_sb = pp.tile([128, S], BF16)        # kv head dup in both halves
        vtok = pp.tile([128, NKT, 65], BF16)   # token-major v + ones col
        nc.vector.memset(vtok, 0.0)
        for kt in range(NKT):
            nc.vector.memset(vtok[:, kt, 64:65], 1.0)
        g_sb = pp.tile([128, NIF, TSH], BF16)  # gathered out (post-A2A)

        # 8 PSUM bank tags: proj pq0/pq1/pkv/ptp, attn ao0/ao1/as0/as1
        DT = ["as0", "as1", "ao0", "ao1"]     # phase-D rotation

        def rw_chain(pool, lg_ps, ngrp, ntok, tag):
            """Batched router softmax.

            lg_ps: [8*ngrp, ntok] f32 logits view (PSUM, any base).
            Returns DRAM handle rw_dr [8*ngrp, ntok] f32 holding softmax
            weights; caller DMA-broadcasts rows into SBUF.
            """
            nch = ntok // 128
            nr = 8 * ngrp
            lgT = pool.tile([nr, ntok], F32, name="lgT", tag="lgT", bufs=2)
            nc.vector.tensor_copy(lgT, lg_ps)
            tp_ps = ps.tile([128, nch * nr], F32, name="tp_ps", tag="ptp")
            for c in range(nch):
                nc.tensor.transpose(tp_ps[:, nr * c:nr * c + nr],
                                    lgT[:, 128 * c:128 * c + 128],
                                    ident_f[0:nr, 0:nr])
            lgtok = pool.tile([128, nch, ngrp, 8], F32, name="lgtok",
                              tag="lgtok", bufs=2)
            nc.vector.tensor_copy(
                lgtok, tp_ps.rearrange("p (n g e) -> p n g e", g=ngrp, e=8))
            mx = pool.tile([128, nch, ngrp], F32, name="mx", tag="mx", bufs=2)
            nc.vector.tensor_reduce(mx, lgtok, axis=AX.X, op=AluOpType.max)
            lgs = pool.tile([128, nch, ngrp, 8], F32, name="lgs", tag="lgs",
                            bufs=2)
            nc.vector.tensor_tensor(
                lgs, lgtok,
                mx.unsqueeze(3).broadcast_to([128, nch, ngrp, 8]),
                AluOpType.subtract)
            ex = pool.tile([128, nch, ngrp, 8], F32, name="ex", tag="ex",
                           bufs=2)
            nc.scalar.activation(ex, lgs, AF.Exp)
            sm = pool.tile([128, nch, ngrp], F32, name="sm", tag="sm", bufs=2)
            nc.vector.tensor_reduce(sm, ex, axis=AX.X, op=AluOpType.add)
            rc = pool.tile([128, nch, ngrp], F32, name="rc", tag="rc", bufs=2)
            nc.vector.reciprocal(rc, sm)
            rw = pool.tile([128, nch, ngrp, 8], F32, name="rw", tag="rw",
                           bufs=2)
            nc.vector.tensor_tensor(
                rw, ex, rc.unsqueeze(3).broadcast_to([128, nch, ngrp, 8]),
                AluOpType.mult)
            rwT_ps = ps.tile([nr, ntok], F32, name="rwT_ps", tag="ptp")
            for c in range(nch):
                nc.tensor.transpose(rwT_ps[:, 128 * c:128 * c + 128],
                                    rw[:, c, :, :], ident_f[:, 0:128])
            rwT = pool.tile([nr, ntok], F32, name="rwT", tag="rwT", bufs=2)
            nc.vector.tensor_copy(rwT, rwT_ps)
            rw_dr = pd.tile([nr, ntok], F32, name="rw_dr", tag=tag, bufs=2)
            nc.scalar.dma_start(out=rw_dr, in_=rwT)
            return rw_dr

        def rw_bcast(pool, rw_dr, grp, ntok, out_base, name):
            """Broadcast rows of group `grp` (8 rows) to 64 partitions
            (row r*8+e), into partitions [out_base, out_base+64)."""
            rwx = pool.tile([out_base + 64, ntok], F32, name=name, tag=name,
                            bufs=2)
            nc.scalar.dma_start(
                out=rwx[out_base:out_base + 64, :],
                in_=bass.AP(tensor=rw_dr.tensor,
                            offset=rw_dr.offset + 8 * grp * ntok,
                            ap=[[0, R], [ntok, R], [1, ntok]]))
            return rwx

        # =================== main interleaved loop ===================
        pA = ctx.enter_context(tc.tile_pool(name="pA", bufs=1))
        pC = ctx.enter_context(tc.tile_pool(name="pC", bufs=1))

        def proj_units(i):
            """Emission thunks for the projections of token block i."""
            tsl = slice(i * 512, (i + 1) * 512)
            st_ = {}
            units = []

            def t_dma():
                xq = pA.tile([128, NIF, 512], BF16, name="xq", tag="xq",
                             bufs=2)
                nc.scalar.dma_start(
                    out=xq,
                    in_=xT.rearrange("(n p) t -> p n t", p=128)[:, :, tsl])
                st_["xq"] = xq
                csl = pA.tile([128, 512], BF16, name="csl", tag="csl", bufs=2)
                nc.gpsimd.dma_start(out=csl, in_=cs[:, tsl])
                ssl = pA.tile([128, 512], BF16, name="ssl", tag="ssl", bufs=2)
                nc.gpsimd.dma_start(out=ssl, in_=sn[:, tsl])
                m01_t = pA.tile([128, 4, 512], BF16, name="m01", tag="m01",
                                bufs=2)
                nc.gpsimd.dma_start(
                    out=m01_t, in_=m01.rearrange("(n p) f -> p n f", p=128)
                    [:, 4 * i:4 * i + 4, :])
                st_["cs"], st_["sn"], st_["m01"] = csl, ssl, m01_t
                L1_ps = ps.tile([128, 512], F32, name="L1", tag="pq0")
                L2_ps = ps.tile([88, 512], F32, name="L2", tag="pq1")
                st_["L1"], st_["L2"] = L1_ps, L2_ps
            units.append(t_dma)

            def t_L(k):
                st, sp = k == 0, k == NIF - 1
                nc.tensor.matmul(st_["L1"], l1_sb[:, k, :],
                                 st_["xq"][:, k, :], start=st, stop=sp)
                nc.tensor.matmul(st_["L2"], l2_sb[:, k, :],
                                 st_["xq"][:, k, :], start=st, stop=sp)
            for k in range(NIF):
                units.append(lambda k=k: t_L(k))

            def t_lg():
                # copy h parts to SBUF (frees the L banks for qe/qo), then
                # run the batched router-softmax chain
                hA = pA.tile([128, 512], BF16, name="hA", tag="hA", bufs=2)
                nc.vector.tensor_copy(hA, st_["L1"])
                hV = pA.tile([64, 512], BF16, name="hV", tag="hV", bufs=2)
                nc.vector.tensor_copy(hV, st_["L2"][0:64, :])
                st_["hA"], st_["hV"] = hA, hV
                st_["rw_dr"] = rw_chain(pA, st_["L2"][64:88, :], 3, 512,
                                        "rwqkv")
            units.append(t_lg)

            def t_qalloc():
                st_["qe"] = ps.tile([128, 512], F32, name="qe", tag="pq0")
                st_["qo"] = ps.tile([128, 512], F32, name="qo", tag="pq1")
                st_["kv"] = ps.tile([128, 512], F32, name="kv", tag="pkv")
            units.append(t_qalloc)

            def t_Q(k):
                rhs = st_["xq"][:, k, :]
                st = k == 0
                nc.tensor.matmul(st_["qe"], wqT_sb[:, k, 0:128], rhs,
                                 start=st, stop=False)
                nc.tensor.matmul(st_["qo"], wqT_sb[:, k, 128:256], rhs,
                                 start=st, stop=False)
                nc.tensor.matmul(st_["kv"], wkv_sb[:, k, :], rhs,
                                 start=st, stop=False)
            for k in range(NIF):
                units.append(lambda k=k: t_Q(k))

            def t_badd():
                rwx_q = rw_bcast(pA, st_["rw_dr"], 0, 512, 0, "rwx_q")
                rwx_k = rw_bcast(pA, st_["rw_dr"], 1, 512, 64, "rwx_k")
                rwx_v = rw_bcast(pA, st_["rw_dr"], 2, 512, 0, "rwx_v")
                hp_q = pA.tile([64, 512], BF16, name="hp_q", tag="hp_q",
                               bufs=2)
                nc.vector.tensor_tensor(hp_q, st_["hA"][0:64, :], rwx_q,
                                        AluOpType.mult)
                hp_kv = pA.tile([128, 512], BF16, name="hp_kv", tag="hp_kv",
                                bufs=2)
                nc.vector.tensor_tensor(hp_kv[64:128, :],
                                        st_["hA"][64:128, :],
                                        rwx_k[64:128, :], AluOpType.mult)
                nc.vector.tensor_tensor(hp_kv[0:64, :], st_["hV"], rwx_v,
                                        AluOpType.mult)
                nc.tensor.matmul(st_["qe"], bqe_sb, hp_q, start=False,
                                 stop=True)
                nc.tensor.matmul(st_["qo"], bqo_sb, hp_q, start=False,
                                 stop=True)
                nc.tensor.matmul(st_["kv"], bkv_sb, hp_kv, start=False,
                                 stop=True)
            units.append(t_badd)

            def t_rope():
                csl, ssl = st_["cs"], st_["sn"]
                qe_ps, qo_ps = st_["qe"], st_["qo"]
                tm1 = pA.tile([128, 512], F32, name="tm1", tag="tm1", bufs=2)
                tm2 = pA.tile([128, 512], F32, name="tm2", tag="tm2", bufs=2)
                qre = pA.tile([128, 512], BF16, name="qre", tag="qre", bufs=2)
                qro = pA.tile([128, 512], BF16, name="qro", tag="qro", bufs=2)
                nc.vector.tensor_tensor(tm1, qe_ps, csl, AluOpType.mult)
                nc.vector.tensor_tensor(tm2, qo_ps, ssl, AluOpType.mult)
                nc.vector.tensor_tensor(qre, tm1, tm2, AluOpType.subtract)
                nc.vector.tensor_tensor(tm1, qe_ps, ssl, AluOpType.mult)
                nc.vector.tensor_tensor(tm2, qo_ps, csl, AluOpType.mult)
                nc.vector.tensor_tensor(qro, tm1, tm2, AluOpType.add)
                for h in range(QH):
                    page, half = h // 2, h % 2
                    nc.scalar.dma_start(
                        out=qh_sb[64 * half:64 * half + 32, page, tsl],
                        in_=qre[32 * h:32 * h + 32, :])
                    nc.scalar.dma_start(
                        out=qh_sb[64 * half + 32:64 * half + 64, page, tsl],
                        in_=qro[32 * h:32 * h + 32, :])
            units.append(t_rope)

            def t_krv():
                csl, ssl, kv_ps = st_["cs"], st_["sn"], st_["kv"]
                kpre = pA.tile([32, 2, 512], F32, name="kpre", tag="kpre",
                               bufs=2)
                nc.vector.tensor_copy(kpre[:, 0, :], kv_ps[0:32, :])
                nc.vector.tensor_copy(kpre[:, 1, :], kv_ps[32:64, :])
                krot = pA.tile([32, 2, 512], BF16, name="krot", tag="krot",
                               bufs=2)
                te = pA.tile([32, 512], F32, name="te", tag="te", bufs=2)
                to = pA.tile([32, 512], F32, name="to", tag="to", bufs=2)
                nc.vector.tensor_tensor(te, kpre[:, 0, :], csl[0:32, :],
                                        AluOpType.mult)
                nc.vector.tensor_tensor(to, kpre[:, 1, :], ssl[0:32, :],
                                        AluOpType.mult)
                nc.vector.tensor_tensor(krot[:, 0, :], te, to,
                                        AluOpType.subtract)
                nc.vector.tensor_tensor(te, kpre[:, 0, :], ssl[0:32, :],
                                        AluOpType.mult)
                nc.vector.tensor_tensor(to, kpre[:, 1, :], csl[0:32, :],
                                        AluOpType.mult)
                nc.vector.tensor_tensor(krot[:, 1, :], te, to, AluOpType.add)
                for half in range(2):
                    nc.scalar.dma_start(
                        out=kh_sb[64 * half:64 * half + 32, tsl],
                        in_=krot[:, 0, :])
                    nc.scalar.dma_start(
                        out=kh_sb[64 * half + 32:64 * half + 64, tsl],
                        in_=krot[:, 1, :])
                vT_t = pA.tile([64, 512], BF16, name="vT", tag="vT", bufs=2)
                nc.vector.tensor_copy(vT_t, kv_ps[64:128, :])
                for j in range(4):
                    v_ps = ps.tile([128, 64], BF16, name="v_ps", tag="ptp")
                    nc.tensor.transpose(v_ps,
                                        vT_t[:, 128 * j:128 * j + 128],
                                        ident_b[0:64, 0:64])
                    nc.vector.tensor_copy(vtok[:, 4 * i + j, 0:64], v_ps)
            units.append(t_krv)
            return units, st_

        def attn_units(qb, m01_t):
            """Emission thunks for query block qb: 2 passes x 2 heads."""
            active = [kt for kt in range(NKT) if mask_cls[kt, qb] != M_SKIP]
            assert active
            units = []
            for p in range(2):
                stp = {}

                def t_oalloc(p=p, stp=stp):
                    stp["o"] = [ps.tile([65, 512], F32, name="outp%d" % hh,
                                        tag="ao%d" % hh)
                                for hh in range(2)]
                    stp["prev"] = None
                units.append(t_oalloc)

                def t_grp(n_kt, kt, p=p, stp=stp):
                    ksl = slice(128 * kt, 128 * kt + 128)
                    madd = mask_cls[kt, qb] == M_ADD
                    off = 128 * (kt - active[-4]) if madd else 0
                    osl = slice(qb * 512 + off, (qb + 1) * 512)
                    scs = []
                    for hh in range(2):
                        sc = ps.tile([128, 512], F32, name="sc%d" % hh,
                                     tag="as%d" % hh)
                        nc.tensor.matmul(
                            sc[:, off:], kh_sb[64 * hh:64 * hh + 64, ksl],
                            qh_sb[64 * hh:64 * hh + 64, p, osl],
                            start=True, stop=True,
                            tile_position=(64 * hh, 0))
                        scs.append(sc)
                    if stp["prev"] is not None:
                        pkt, pprs, poff = stp["prev"]
                        for hh in range(2):
                            nc.tensor.matmul(
                                stp["o"][hh][:, poff:], vtok[:, pkt, :],
                                pprs[hh][:, poff:],
                                start=(pkt == active[0]), stop=False)
                    prs = []
                    for hh in range(2):
                        pr = pC.tile([128, 512], BF16, name="pr", tag="pr",
                                     bufs=8)
                        nc.scalar.activation(pr[:, off:], scs[hh][:, off:],
                                             AF.Exp, scale=LN2)
                        if madd:
                            mi = kt - active[-4]
                            nc.vector.tensor_tensor(pr[:, off:], pr[:, off:],
                                                    m01_t[:, mi, off:],
                                                    AluOpType.mult)
                        prs.append(pr)
                    stp["prev"] = (kt, prs, off)
                for n_kt, kt in enumerate(active):
                    units.append(lambda f=t_grp, n_kt=n_kt, kt=kt:
                                 f(n_kt, kt))

                def t_ship(p=p, stp=stp):
                    pkt, pprs, poff = stp["prev"]
                    for hh in range(2):
                        nc.tensor.matmul(stp["o"][hh][:, poff:],
                                         vtok[:, pkt, :], pprs[hh][:, poff:],
                                         start=(pkt == active[0]), stop=True)
                    for hh in range(2):
                        h = 2 * p + hh
                        on65 = pC.tile([65, 512], BF16, name="on65",
                                       tag="on65", bufs=4)
                        nc.vector.tensor_copy(on65, stp["o"][hh])
                        for half in range(2):
                            hsl = slice(256 * half, 256 * half + 256)
                            nc.sync.dma_start(
                                out=cc_in[2 * qb + half,
                                          64 * h:64 * h + 64, :],
                                in_=on65[0:64, hsl])
                            nc.sync.dma_start(
                                out=cc_in[2 * qb + half, QF + h, :],
                                in_=on65[64:65, hsl])
                units.append(t_ship)
            return units

        def merge(P, A):
            n, m = len(P), len(A)
            i = j = 0
            while i < n or j < m:
                if j >= m or (i < n and i * m <= j * n):
                    P[i]()
                    i += 1
                else:
                    A[j]()
                    j += 1

        prev_m01 = None
        for it in range(NQB + 1):
            P, st_ = (proj_units(it) if it < NQB else ([], None))
            A = attn_units(it - 1, prev_m01) if it >= 1 else []
            merge(P, A)
            prev_m01 = st_["m01"] if st_ is not None else None

        # =================== A2A + o-proj ===================
        nc.gpsimd.collective_compute(
            "AllToAll", AluOpType.bypass, ins=[cc_in[:]], outs=[cc_out[:]],
            replica_groups=[list(range(NCORES))])

        g_v = g_sb.rearrange("p (c n) t -> p c n t", n=2)
        for n in range(2):
            nc.sync.dma_start(
                out=g_v[:, :, n, :],
                in_=cc_out[:, 128 * n:128 * n + 128, :]
                    .rearrange("c p t -> p c t"))
        den_all = pC.tile([32, TSH], BF16, name="den_all")
        for cb in range(NCORES):
            nc.sync.dma_start(out=den_all[QH * cb:QH * cb + QH, :],
                              in_=cc_out[cb, QF:QF + QH, :])
        rec32 = pC.tile([32, TSH], F32, name="rec32")
        nc.vector.reciprocal(rec32, den_all)
        g_n = pC.tile([128, NIF, TSH], BF16, name="g_n")
        for k in range(NIF):
            rb_ps = ps.tile([128, TSH], F32, name="rb_ps",
                            tag=DT[k % 4])
            nc.tensor.matmul(rb_ps, sel_sb[:, 128 * k:128 * k + 128],
                             rec32, start=True, stop=True)
            nc.vector.tensor_tensor(g_n[:, k, :], g_sb[:, k, :], rb_ps,
                                    AluOpType.mult)

        ho_ps = ps.tile([72, TSH], F32, name="ho", tag="pq0")
        for k in range(NIF):
            nc.tensor.matmul(ho_ps, ao_sb[:, k, :], g_n[:, k, :],
                             start=(k == 0), stop=(k == NIF - 1))
        rwo_dr = rw_chain(pC, ho_ps[64:72, :], 1, TSH, "rwo")
        rwx_o = rw_bcast(pC, rwo_dr, 0, TSH, 0, "rwx_o")
        hpo = pC.tile([64, TSH], BF16, name="hpo")
        nc.vector.tensor_tensor(hpo, ho_ps[0:64, :], rwx_o, AluOpType.mult)

        for ob in range(4):
            osl = slice(ob * 512, (ob + 1) * 512)
            # reuse the xq slot (same shape, xq dead by phase D)
            wo_sb = pA.tile([128, NIF, 512], BF16, name="xq", tag="xq",
                            bufs=2)
            nc.sync.dma_start(
                out=wo_sb,
                in_=woT.rearrange("(n p) f -> p n f", p=128)[:, :, osl])
            for tt in range(2):
                yp = ps.tile([128, 512], F32, name="yp",
                             tag=["pq1", "pkv"][tt])
                for k in range(NIF):
                    nc.tensor.matmul(yp, g_n[:, k, 128 * tt:128 * tt + 128],
                                     wo_sb[:, k, :], start=(k == 0),
                                     stop=False)
                nc.tensor.matmul(yp, hpo[:, 128 * tt:128 * tt + 128],
                                 bo_sb[:, osl], start=False, stop=True)
                yt = pC.tile([128, 512], F32, name="yt", tag="yt", bufs=2)
                nc.vector.tensor_copy(yt, yp)
                nc.sync.dma_start(out=y[128 * tt:128 * tt + 128, osl],
                                  in_=yt)


# ======================= host side =======================

_CACHE = {}


def _prep_inputs(x, mask, freqs_cos, freqs_sin, wq, wk, wv, wo,
                 lq_router, lq_A, lq_B, lk_router, lk_A, lk_B,
                 lv_router, lv_A, lv_B, lo_router, lo_A, lo_B):
    scale = float(np.log2(np.e)) / np.sqrt(HD)  # log2e folded: exp via 2^x
    x = _f32(np.asarray(x)).reshape(S, D)
    maskf = _f32(np.asarray(mask)).reshape(S, S)
    maskT = np.maximum(maskf, MASK_NEG).T.copy()
    mask_cls = classify_mask(maskT)

    xTb = _bf(x.T)
    cs4 = _bf(np.tile(_f32(freqs_cos).T, (4, 1)))      # [128, S]
    sn4 = _bf(np.tile(_f32(freqs_sin).T, (4, 1)))
    woTb = _bf(_f32(wo).T)

    # 0/1 mask tiles for the diagonal (M_ADD) blocks, stacked [16*128, 512]
    m01 = np.zeros((NQB * 4 * 128, 512), dtype=np.float32)
    for qb in range(NQB):
        adds = [kt for kt in range(NKT) if mask_cls[kt, qb] == M_ADD]
        for j, kt in enumerate(adds[-4:]):
            blk = maskT[128 * kt:128 * kt + 128,
                        512 * qb:512 * qb + 512]
            m01[128 * (4 * qb + j):128 * (4 * qb + j + 1)] = (blk == 0.0)

    ao_p = _bf(np.concatenate([_a64(_f32(lo_A)), _f32(lo_router).T], axis=1))
    bo_f = _bf(_b_flat(_f32(lo_B), SCALING))

    sel_m = np.zeros((H, NIF * 128), dtype=np.float32)
    for k in range(NIF):
        for p in range(128):
            sel_m[2 * k + p // 64, 128 * k + p] = 1.0
    shared = dict(xT=xTb, cs=cs4, sn=sn4, woT=woTb, m01=_bf(m01),
                  ao=ao_p, bo=bo_f, sel=sel_m)

    l1_p = _bf(np.concatenate([_a64(_f32(lq_A)), _a64(_f32(lk_A))], axis=1))
    l2_p = _bf(np.concatenate([_a64(_f32(lv_A)), _f32(lq_router).T,
                               _f32(lk_router).T, _f32(lv_router).T], axis=1))

    wqf, wkf, wvf = _f32(wq), _f32(wk), _f32(wv)
    lqB, lkB, lvB = _f32(lq_B), _f32(lk_B), _f32(lv_B)

    in_maps = []
    for c in range(NCORES):
        wq_c = wqf[c * QF:(c + 1) * QF] * scale
        wqT_c = np.concatenate([wq_c[IDX_QE].T, wq_c[IDX_QO].T], axis=1)
        wk_c = wkf[c * KF:(c + 1) * KF][IDX_K]
        wv_c = wvf[c * KF:(c + 1) * KF]
        wkv_c = np.concatenate([wk_c.T, wv_c.T], axis=1)
        bq_c = _b_flat(lqB[:, c * QF:(c + 1) * QF, :], SCALING * scale)
        bk_c = _b_flat(lkB[:, c * KF:(c + 1) * KF, :][:, IDX_K, :], SCALING)
        bv_c = _b_flat(lvB[:, c * KF:(c + 1) * KF, :], SCALING)
        # hp_kv rows 0:64 = h_v*rw_v, rows 64:128 = h_k*rw_k;
        # kv out rows 0:64 = k-proj, 64:128 = v-proj
        bkv_c = np.zeros((128, 128), dtype=np.float32)
        bkv_c[64:128, 0:64] = bk_c
        bkv_c[0:64, 64:128] = bv_c
        m = dict(shared)
        m.update(wqT=_bf(wqT_c), wkv=_bf(wkv_c), l1=l1_p, l2=l2_p,
                 bqe=_bf(bq_c[:, IDX_QE]), bqo=_bf(bq_c[:, IDX_QO]),
                 bkv=_bf(bkv_c))
        in_maps.append(m)
    return in_maps, mask_cls


def get_graph(mask_cls):
    key = mask_cls.tobytes()
    if key not in _CACHE:
        _CACHE[key] = build(mask_cls)
    return _CACHE[key]


def kernel(x, start_pos, mask, freqs_cos, freqs_sin, wq, wk, wv, wo,
           lq_router, lq_A, lq_B, lk_router, lk_A, lk_B,
           lv_router, lv_A, lv_B, lo_router, lo_A, lo_B,
           _trace=False):
    from concourse.bass_utils import run_bass_kernel_spmd
    in_maps, mask_cls = _prep_inputs(
        x, mask, freqs_cos, freqs_sin, wq, wk, wv, wo,
        lq_router, lq_A, lq_B, lk_router, lk_A, lk_B,
        lv_router, lv_A, lv_B, lo_router, lo_A, lo_B)
    nc = get_graph(mask_cls)
    res = run_bass_kernel_spmd(nc, in_maps, list(range(NCORES)), trace=_trace)
    out = np.concatenate([res.results[c]["y"] for c in range(NCORES)], axis=0)
    out = out.reshape(B, S, H * HD).astype(np.float32)
    if _trace:
        return out, res
    return out
